# revision 20
# baseline (speedup 1.0000x reference)
"""Trainium2 8-core attention kernel (v4).

Problem: B=2, H=16, S=2048, D=64 dense attention, f32 I/O.
Sharding: B*H = 32 head-batches -> 4 heads per NeuronCore (embarrassingly
parallel, no collectives).

Per-core algorithm (transposed score space end-to-end):
  S^T[k, q] = K_dmaj . Q_dmaj      PE matmul, contraction d=64, ROW-TILED
                                   (two concurrent matmuls, row groups 0/64)
  P = exp(S^T / 8)                 3-way exp: ScalarE ACT (exact) +
                                   VectorE & GpSimdE Schraudolph (f32->int16
                                   round(A*s+B) bitcast to bf16)
  outT[d', q] = V'^T @ P           PE matmul, contraction k; V'=[V|ones] so
                                   row 64 = softmax denominator
  outT[:65] -> HBM unnormalized    single DVE copy PSUM->SBUF + DMA; the
                                   host divides rows 0:64 by row 64 and
                                   transposes back (pure layout + one bcast
                                   divide on full output)

Host side reshapes/transposes/casts (layout choices for sharding):
  qt, kt: [4, 128, 2048] bf16 (d on partitions, rows 64:128 duplicate 0:64)
  vp:     [4, 128, 16, 65] bf16 (k%128 on partitions, ones column appended)
  ot:     [4, 65, 2048] f32 (transposed, unnormalized; host divides by row
          64 and transposes to [4, 2048, 64])
"""

import numpy as np
import ml_dtypes

import concourse.bass as bass
import concourse.tile as tile
from concourse import bacc, mybir
from concourse.bass_utils import run_bass_kernel_spmd

B, H, S, D = 2, 16, 2048, 64
NCORES = 8
HPC = (B * H) // NCORES  # heads per core = 4
P = 128
KT = S // P  # 16 k-tiles
SCALE = 1.0 / np.sqrt(D)  # 0.125

# Schraudolph bf16-exp constants: bits16 = round(A*s + B); bitcast -> bf16
SCH_A = float(P * np.log2(np.e) * SCALE)
SCH_B = float(P * 127 - 7.5)


# 2-way exp split (GpSimd cannot read PSUM, so only ACT + DVE can consume
# score tiles). With the epilogue reduced to one copy per chunk, the DVE has
# room for 7/16 tiles per half; spread each engine's tiles through the half
# so the QK stream never throttles on one engine.
_DVE_H0 = {1, 3, 5, 8, 10, 12, 14}
_DVE_H1 = {0, 2, 4, 7, 9, 11, 13}
# last head half1: keep the tail tiles off the DVE, whose queue also carries
# the final output copies.
_DVE_LAST = {1, 3, 5, 8, 10, 12}


def engine_for_tile(kt_i, half, h):
    if h == HPC - 1 and half == 1:
        dve = _DVE_LAST
    elif half == 0:
        dve = _DVE_H0
    else:
        dve = _DVE_H1
    return "dve" if kt_i in dve else "act"


f32 = mybir.dt.float32
bf16 = mybir.dt.bfloat16
i16 = mybir.dt.int16


def emit_loads(nc, pools, aps, h):
    qt, kt, vp, ot = aps
    qk_pool, v_pool, p_pool, epi_pool, ps_s, ps_o = pools
    qt_b = qk_pool.tile([P, S], bf16, tag="qt")
    kt_b = qk_pool.tile([P, S], bf16, tag="kt")
    # split loads so the first QK tile's deps land early
    if h == 0:
        # first QK tile needs kt[:, :128] + qt[:, :1024]: spread those over
        # three queues so they land as early as possible
        nc.sync.dma_start(kt_b[:, :P], kt[h, :, :P])
        nc.scalar.dma_start(qt_b[:, :512], qt[h, :, :512])
        nc.gpsimd.dma_start(qt_b[:, 512:1024], qt[h, :, 512:1024])
        nc.sync.dma_start(kt_b[:, P : S // 2], kt[h, :, P : S // 2])
        nc.sync.dma_start(kt_b[:, S // 2 :], kt[h, :, S // 2 :])
        nc.scalar.dma_start(qt_b[:, 1024:], qt[h, :, 1024:])
    else:
        nc.sync.dma_start(kt_b[:, : S // 2], kt[h, :, : S // 2])
        nc.sync.dma_start(qt_b[:, : S // 2], qt[h, :, : S // 2])
        nc.sync.dma_start(kt_b[:, S // 2 :], kt[h, :, S // 2 :])
        nc.sync.dma_start(qt_b[:, S // 2 :], qt[h, :, S // 2 :])
    v_b = v_pool.tile([P, KT, D + 1], bf16, tag="v")
    nc.sync.dma_start(v_b[:], vp[h])
    p_b = p_pool.tile([P, KT, S], bf16, tag="p")
    return qt_b, kt_b, v_b, p_b


def emit_qk_tile(nc, tc, pools, half, kt_i, qt_b, kt_b, p_b, h=1, grp=None):
    """One [128, 1024] score tile: row-tiled QK pair + exp."""
    qk_pool, v_pool, p_pool, epi_pool, ps_s, ps_o = pools
    q0 = half * 1024
    s_ps = ps_s.tile([P, 1024], f32, tag="s")
    nc.tensor.matmul(
        s_ps[:, 0:512],
        lhsT=kt_b[0:64, kt_i * P : (kt_i + 1) * P],
        rhs=qt_b[0:64, q0 : q0 + 512],
        start=True,
        stop=True,
        tile_position=(0, 0),
    )
    nc.tensor.matmul(
        s_ps[:, 512:1024],
        lhsT=kt_b[64:128, kt_i * P : (kt_i + 1) * P],
        rhs=qt_b[64:128, q0 + 512 : q0 + 1024],
        start=True,
        stop=True,
        tile_position=(64, 0),
    )
    dst = p_b[:, kt_i, q0 : q0 + 1024]
    eng = engine_for_tile(kt_i, half, h)
    if eng == "act":
        nc.scalar.activation(
            dst, s_ps[:], mybir.ActivationFunctionType.Exp, scale=float(SCALE)
        )
    else:
        e = nc.vector if eng == "dve" else nc.gpsimd
        e.tensor_scalar(
            dst.bitcast(i16),
            s_ps[:],
            SCH_A,
            SCH_B,
            mybir.AluOpType.mult,
            mybir.AluOpType.add,
        )


class PVChunk:
    """One 512-wide q-chunk of a head's PV, fed matmul-by-matmul so the MMs
    interleave with the QK stream instead of starving the exp engines."""

    def __init__(self, h, p_b, v_b, qc):
        self.h, self.p_b, self.v_b, self.qc = h, p_b, v_b, qc
        self.o_ps = None
        self.k = 0

    def step(self, nc, tc, pools, aps, n_mm, grp=None):
        qt, kt, vp, ot = aps
        qk_pool, v_pool, p_pool, epi_pool, ps_s, ps_o = pools
        if self.o_ps is None:
            self.o_ps = ps_o.tile([P, 512], f32, tag="o")
        for _ in range(n_mm):
            if self.k >= KT:
                break
            nc.tensor.matmul(
                self.o_ps[: D + 1, :],
                lhsT=self.v_b[:, self.k, :],
                rhs=self.p_b[:, self.k, self.qc * 512 : (self.qc + 1) * 512],
                start=(self.k == 0),
                stop=(self.k == KT - 1),
                skip_group_check=True,
            )
            self.k += 1
        if self.k == KT:
            self.finish(nc, pools, aps)

    def finish(self, nc, pools, aps):
        qt, kt, vp, ot = aps
        qk_pool, v_pool, p_pool, epi_pool, ps_s, ps_o = pools
        ot_sb = epi_pool.tile([D + 1, 512], f32, tag="ot")
        nc.vector.tensor_copy(ot_sb[:], self.o_ps[: D + 1, :])
        nc.sync.dma_start(
            ot[self.h, :, self.qc * 512 : (self.qc + 1) * 512], ot_sb[:]
        )
        self.k = KT + 1  # mark done


def build_nc():
    nc = bacc.Bacc("TRN2", target_bir_lowering=False, debug=False)
    qt = nc.dram_tensor("qt", [HPC, P, S], bf16, kind="ExternalInput").ap()
    kt = nc.dram_tensor("kt", [HPC, P, S], bf16, kind="ExternalInput").ap()
    vp = nc.dram_tensor("vp", [HPC, P, KT, D + 1], bf16, kind="ExternalInput").ap()
    ot = nc.dram_tensor("ot", [HPC, D + 1, S], f32, kind="ExternalOutput").ap()
    aps = (qt, kt, vp, ot)

    with tile.TileContext(nc) as tc:
        with (
            tc.tile_pool(name="qk", bufs=2) as qk_pool,
            tc.tile_pool(name="v", bufs=2) as v_pool,
            tc.tile_pool(name="p", bufs=2) as p_pool,
            tc.tile_pool(name="epi", bufs=3) as epi_pool,
            tc.tile_pool(name="ps_s", bufs=3, space="PSUM") as ps_s,
            tc.tile_pool(name="ps_o", bufs=2, space="PSUM") as ps_o,
        ):
            pools = (qk_pool, v_pool, p_pool, epi_pool, ps_s, ps_o)

            # HAM warm-up: ~3.4us of dummy matmuls during the NEFF preamble
            # so the PE clock is already at 8/8 when the real stream starts.
            # memset on the vector queue: its preamble drains earliest, so
            # the warm-up starts ~2us sooner than gating on gpsimd.
            warm_w = qk_pool.tile([P, P], bf16, tag="warm")
            nc.vector.memset(warm_w[:], 0.0)
            warm_ps = ps_o.tile([P, 512], f32, tag="o")
            for _ in range(30):
                nc.tensor.matmul(
                    warm_ps[:, :P], lhsT=warm_w[:], rhs=warm_w[:],
                    start=True, stop=True,
                )

            # Software pipeline: head h's QK/exp stream is interleaved (at kt
            # granularity) with head h-1's PV chunks so the PE fills its
            # exp-throttled stall slots with PV matmuls.
            # Group counter: each QK triplet / PV run gets a strictly
            # increasing sim-time floor ON ITS PE MATMULS ONLY, so the
            # static scheduler's PE queue order follows the emitted cadence
            # instead of its own latency model (which ping-pongs QK/PV and
            # pays the LDW switch tax ~130x instead of ~90x). Non-PE
            # instructions stay unfloored: strict-FIFO engine queues must
            # keep the scheduler's dependency-aware order.
            grp = [1]

            def next_grp():
                g = grp[0]
                grp[0] += 1
                return g

            # Chunk rotation: head h's half1 interleaves h's own chunks
            # 0/1 (their p-tiles complete with half0); half0 interleaves
            # h-1's chunks 2/3. Only h0-half0 runs without PV filler, and
            # the tail is chunks 2/3 of the last head.
            prev = None
            for h in range(HPC):
                qt_b, kt_b, v_b, p_b = emit_loads(nc, pools, aps, h)
                for half in range(2):
                    jobs = []
                    if half == 0 and prev is not None:
                        jobs.append(PVChunk(h - 1, *prev, 2))
                        jobs.append(PVChunk(h - 1, *prev, 3))
                    if half == 1:
                        jobs.append(PVChunk(h, p_b, v_b, 0))
                        jobs.append(PVChunk(h, p_b, v_b, 1))
                    # Cadence QK*3 / PV*5: QK runs sized to the 3-deep PSUM
                    # cushion, PV runs sized so exp never starves; 7 PV are
                    # held back for the half boundary, where the next QK
                    # triplet waits on the exp backlog of tiles 13-15.
                    qki = 0
                    while qki < KT:
                        g = next_grp()
                        for _ in range(3):
                            if qki < KT:
                                emit_qk_tile(
                                    nc, tc, pools, half, qki, qt_b, kt_b,
                                    p_b, h, grp=g,
                                )
                                qki += 1
                        g = next_grp()
                        n = 5
                        while n > 0 and jobs:
                            take = min(n, KT - jobs[0].k)
                            jobs[0].step(nc, tc, pools, aps, take, grp=g)
                            if jobs[0].k > KT:
                                jobs.pop(0)
                            n -= take
                    g = next_grp()
                    for j in jobs:
                        j.step(nc, tc, pools, aps, KT, grp=g)
                prev = (p_b, v_b)
            g = next_grp()
            for qc in (2, 3):
                PVChunk(HPC - 1, *prev, qc).step(nc, tc, pools, aps, KT, grp=g)

    nc.compile()
    return nc


def shard_inputs(Q, K, V):
    """Full [B,H,S,D] f32 -> per-core input maps (layout + dtype choices)."""
    Qh = np.asarray(Q, dtype=np.float32).reshape(B * H, S, D)
    Kh = np.asarray(K, dtype=np.float32).reshape(B * H, S, D)
    Vh = np.asarray(V, dtype=np.float32).reshape(B * H, S, D)

    in_maps = []
    for c in range(NCORES):
        sl = slice(c * HPC, (c + 1) * HPC)
        qt = np.empty((HPC, P, S), dtype=ml_dtypes.bfloat16)
        kt = np.empty((HPC, P, S), dtype=ml_dtypes.bfloat16)
        qt[:, :D, :] = Qh[sl].transpose(0, 2, 1).astype(ml_dtypes.bfloat16)
        kt[:, :D, :] = Kh[sl].transpose(0, 2, 1).astype(ml_dtypes.bfloat16)
        qt[:, D:, :] = qt[:, :D, :]  # duplicate for row-group 64-127
        kt[:, D:, :] = kt[:, :D, :]
        vp = np.ones((HPC, S, D + 1), dtype=np.float32)
        vp[:, :, :D] = Vh[sl]
        # [h, (kt p), d] -> [h, p, kt, d']
        vp = (
            vp.reshape(HPC, KT, P, D + 1)
            .transpose(0, 2, 1, 3)
            .astype(ml_dtypes.bfloat16)
        )
        in_maps.append({"qt": np.ascontiguousarray(qt),
                        "kt": np.ascontiguousarray(kt),
                        "vp": np.ascontiguousarray(vp)})
    return in_maps


_NC_CACHE = None


def unshard_outputs(res):
    out = np.empty((B * H, S, D), dtype=np.float32)
    for c in range(NCORES):
        o = res.results[c]["ot"]  # [HPC, D+1, S] unnormalized, transposed
        out[c * HPC : (c + 1) * HPC] = (
            o[:, :D, :] / o[:, D : D + 1, :]
        ).transpose(0, 2, 1)
    return out.reshape(B, H, S, D)


def kernel(Q, K, V):
    global _NC_CACHE
    if _NC_CACHE is None:
        _NC_CACHE = build_nc()
    nc = _NC_CACHE
    in_maps = shard_inputs(Q, K, V)
    res = run_bass_kernel_spmd(nc, in_maps, core_ids=list(range(NCORES)))
    return unshard_outputs(res)


if __name__ == "__main__":
    nc = build_nc()
    print("compiled OK")


# revision 21
# speedup vs baseline: 1.0101x; 1.0101x over previous
"""Trainium2 8-core attention kernel (v4).

Problem: B=2, H=16, S=2048, D=64 dense attention, f32 I/O.
Sharding: B*H = 32 head-batches -> 4 heads per NeuronCore (embarrassingly
parallel, no collectives).

Per-core algorithm (transposed score space end-to-end):
  S^T[k, q] = K_dmaj . Q_dmaj      PE matmul, contraction d=64, ROW-TILED
                                   (two concurrent matmuls, row groups 0/64)
  P = exp(S^T / 8)                 3-way exp: ScalarE ACT (exact) +
                                   VectorE & GpSimdE Schraudolph (f32->int16
                                   round(A*s+B) bitcast to bf16)
  outT[d', q] = V'^T @ P           PE matmul, contraction k; V'=[V|ones] so
                                   row 64 = softmax denominator
  outT[:65] -> HBM unnormalized    single DVE copy PSUM->SBUF + DMA; the
                                   host divides rows 0:64 by row 64 and
                                   transposes back (pure layout + one bcast
                                   divide on full output)

Host side reshapes/transposes/casts (layout choices for sharding):
  qt, kt: [4, 128, 2048] bf16 (d on partitions, rows 64:128 duplicate 0:64)
  vp:     [4, 128, 16, 65] bf16 (k%128 on partitions, ones column appended)
  ot:     [4, 65, 2048] f32 (transposed, unnormalized; host divides by row
          64 and transposes to [4, 2048, 64])
"""

import numpy as np
import ml_dtypes

import concourse.bass as bass
import concourse.tile as tile
from concourse import bacc, mybir
from concourse.bass_utils import run_bass_kernel_spmd

B, H, S, D = 2, 16, 2048, 64
NCORES = 8
HPC = (B * H) // NCORES  # heads per core = 4
P = 128
KT = S // P  # 16 k-tiles
SCALE = 1.0 / np.sqrt(D)  # 0.125

# Schraudolph bf16-exp constants: bits16 = round(A*s + B); bitcast -> bf16
SCH_A = float(P * np.log2(np.e) * SCALE)
SCH_B = float(P * 127 - 7.5)


# 2-way exp split (GpSimd cannot read PSUM, so only ACT + DVE can consume
# score tiles). With the epilogue reduced to one copy per chunk, the DVE has
# room for 7/16 tiles per half; spread each engine's tiles through the half
# so the QK stream never throttles on one engine.
_DVE_H0 = {1, 3, 5, 8, 10, 12, 14}
_DVE_H1 = {0, 2, 4, 7, 9, 11, 13}
# last head half1: keep the tail tiles off the DVE, whose queue also carries
# the final output copies.
_DVE_LAST = {1, 3, 5, 8, 10, 12}


def engine_for_tile(kt_i, half, h):
    if h == HPC - 1 and half == 1:
        dve = _DVE_LAST
    elif half == 0:
        dve = _DVE_H0
    else:
        dve = _DVE_H1
    return "dve" if kt_i in dve else "act"


f32 = mybir.dt.float32
bf16 = mybir.dt.bfloat16
i16 = mybir.dt.int16


def emit_loads(nc, pools, aps, h):
    qt, kt, vp, ot = aps
    qk_pool, v_pool, p_pool, epi_pool, ps_s, ps_o = pools
    qt_b = qk_pool.tile([P, S], bf16, tag="qt")
    kt_b = qk_pool.tile([P, S], bf16, tag="kt")
    # split loads so the first QK tile's deps land early
    if h == 0:
        # first QK tile needs kt[:, :128] + qt[:, :1024]: issue those from
        # the sync queue (shortest preamble — scalar sits behind the ACT
        # table load, gpsimd behind a ~3us barrier) so they land earliest.
        nc.sync.dma_start(kt_b[:, :P], kt[h, :, :P])
        nc.sync.dma_start(qt_b[:, :512], qt[h, :, :512])
        nc.sync.dma_start(qt_b[:, 512:1024], qt[h, :, 512:1024])
        nc.gpsimd.dma_start(kt_b[:, P : S // 2], kt[h, :, P : S // 2])
        nc.scalar.dma_start(kt_b[:, S // 2 :], kt[h, :, S // 2 :])
        nc.scalar.dma_start(qt_b[:, 1024:], qt[h, :, 1024:])
    else:
        nc.sync.dma_start(kt_b[:, : S // 2], kt[h, :, : S // 2])
        nc.sync.dma_start(qt_b[:, : S // 2], qt[h, :, : S // 2])
        nc.sync.dma_start(kt_b[:, S // 2 :], kt[h, :, S // 2 :])
        nc.sync.dma_start(qt_b[:, S // 2 :], qt[h, :, S // 2 :])
    v_b = v_pool.tile([P, KT, D + 1], bf16, tag="v")
    nc.sync.dma_start(v_b[:], vp[h])
    p_b = p_pool.tile([P, KT, S], bf16, tag="p")
    return qt_b, kt_b, v_b, p_b


def emit_qk_tile(nc, tc, pools, half, kt_i, qt_b, kt_b, p_b, h=1, grp=None):
    """One [128, 1024] score tile: row-tiled QK pair + exp."""
    qk_pool, v_pool, p_pool, epi_pool, ps_s, ps_o = pools
    q0 = half * 1024
    s_ps = ps_s.tile([P, 1024], f32, tag="s")
    nc.tensor.matmul(
        s_ps[:, 0:512],
        lhsT=kt_b[0:64, kt_i * P : (kt_i + 1) * P],
        rhs=qt_b[0:64, q0 : q0 + 512],
        start=True,
        stop=True,
        tile_position=(0, 0),
    )
    nc.tensor.matmul(
        s_ps[:, 512:1024],
        lhsT=kt_b[64:128, kt_i * P : (kt_i + 1) * P],
        rhs=qt_b[64:128, q0 + 512 : q0 + 1024],
        start=True,
        stop=True,
        tile_position=(64, 0),
    )
    dst = p_b[:, kt_i, q0 : q0 + 1024]
    eng = engine_for_tile(kt_i, half, h)
    if eng == "act":
        nc.scalar.activation(
            dst, s_ps[:], mybir.ActivationFunctionType.Exp, scale=float(SCALE)
        )
    else:
        e = nc.vector if eng == "dve" else nc.gpsimd
        e.tensor_scalar(
            dst.bitcast(i16),
            s_ps[:],
            SCH_A,
            SCH_B,
            mybir.AluOpType.mult,
            mybir.AluOpType.add,
        )


class PVChunk:
    """One 512-wide q-chunk of a head's PV, fed matmul-by-matmul so the MMs
    interleave with the QK stream instead of starving the exp engines."""

    def __init__(self, h, p_b, v_b, qc):
        self.h, self.p_b, self.v_b, self.qc = h, p_b, v_b, qc
        self.o_ps = None
        self.k = 0

    def step(self, nc, tc, pools, aps, n_mm, grp=None):
        qt, kt, vp, ot = aps
        qk_pool, v_pool, p_pool, epi_pool, ps_s, ps_o = pools
        if self.o_ps is None:
            self.o_ps = ps_o.tile([P, 512], f32, tag="o")
        for _ in range(n_mm):
            if self.k >= KT:
                break
            nc.tensor.matmul(
                self.o_ps[: D + 1, :],
                lhsT=self.v_b[:, self.k, :],
                rhs=self.p_b[:, self.k, self.qc * 512 : (self.qc + 1) * 512],
                start=(self.k == 0),
                stop=(self.k == KT - 1),
                skip_group_check=True,
            )
            self.k += 1
        if self.k == KT:
            self.finish(nc, pools, aps)

    def finish(self, nc, pools, aps):
        qt, kt, vp, ot = aps
        qk_pool, v_pool, p_pool, epi_pool, ps_s, ps_o = pools
        ot_sb = epi_pool.tile([D + 1, 512], f32, tag="ot")
        nc.vector.tensor_copy(ot_sb[:], self.o_ps[: D + 1, :])
        nc.sync.dma_start(
            ot[self.h, :, self.qc * 512 : (self.qc + 1) * 512], ot_sb[:]
        )
        self.k = KT + 1  # mark done


def build_nc():
    nc = bacc.Bacc("TRN2", target_bir_lowering=False, debug=False)
    qt = nc.dram_tensor("qt", [HPC, P, S], bf16, kind="ExternalInput").ap()
    kt = nc.dram_tensor("kt", [HPC, P, S], bf16, kind="ExternalInput").ap()
    vp = nc.dram_tensor("vp", [HPC, P, KT, D + 1], bf16, kind="ExternalInput").ap()
    ot = nc.dram_tensor("ot", [HPC, D + 1, S], f32, kind="ExternalOutput").ap()
    aps = (qt, kt, vp, ot)

    with tile.TileContext(nc) as tc:
        with (
            tc.tile_pool(name="qk", bufs=2) as qk_pool,
            tc.tile_pool(name="v", bufs=2) as v_pool,
            tc.tile_pool(name="p", bufs=2) as p_pool,
            tc.tile_pool(name="epi", bufs=3) as epi_pool,
            tc.tile_pool(name="ps_s", bufs=3, space="PSUM") as ps_s,
            tc.tile_pool(name="ps_o", bufs=2, space="PSUM") as ps_o,
        ):
            pools = (qk_pool, v_pool, p_pool, epi_pool, ps_s, ps_o)

            # HAM warm-up: ~3.4us of dummy matmuls during the NEFF preamble
            # so the PE clock is already at 8/8 when the real stream starts.
            # memset on the vector queue: its preamble drains earliest, so
            # the warm-up starts ~2us sooner than gating on gpsimd.
            warm_w = qk_pool.tile([P, P], bf16, tag="warm")
            nc.vector.memset(warm_w[:], 0.0)
            warm_ps = ps_o.tile([P, 512], f32, tag="o")
            for _ in range(30):
                nc.tensor.matmul(
                    warm_ps[:, :P], lhsT=warm_w[:], rhs=warm_w[:],
                    start=True, stop=True,
                )

            # Software pipeline: head h's QK/exp stream is interleaved (at kt
            # granularity) with head h-1's PV chunks so the PE fills its
            # exp-throttled stall slots with PV matmuls.
            # Group counter: each QK triplet / PV run gets a strictly
            # increasing sim-time floor ON ITS PE MATMULS ONLY, so the
            # static scheduler's PE queue order follows the emitted cadence
            # instead of its own latency model (which ping-pongs QK/PV and
            # pays the LDW switch tax ~130x instead of ~90x). Non-PE
            # instructions stay unfloored: strict-FIFO engine queues must
            # keep the scheduler's dependency-aware order.
            grp = [1]

            def next_grp():
                g = grp[0]
                grp[0] += 1
                return g

            # Chunk rotation: head h's half1 interleaves h's own chunks
            # 0/1 (their p-tiles complete with half0); half0 interleaves
            # h-1's chunks 2/3. Only h0-half0 runs without PV filler, and
            # the tail is chunks 2/3 of the last head.
            prev = None
            for h in range(HPC):
                qt_b, kt_b, v_b, p_b = emit_loads(nc, pools, aps, h)
                for half in range(2):
                    jobs = []
                    if half == 0 and prev is not None:
                        jobs.append(PVChunk(h - 1, *prev, 2))
                        jobs.append(PVChunk(h - 1, *prev, 3))
                    if half == 1:
                        jobs.append(PVChunk(h, p_b, v_b, 0))
                        jobs.append(PVChunk(h, p_b, v_b, 1))
                    # Cadence QK*3 / PV*5: QK runs sized to the 3-deep PSUM
                    # cushion, PV runs sized so exp never starves; 7 PV are
                    # held back for the half boundary, where the next QK
                    # triplet waits on the exp backlog of tiles 13-15.
                    qki = 0
                    while qki < KT:
                        g = next_grp()
                        for _ in range(3):
                            if qki < KT:
                                emit_qk_tile(
                                    nc, tc, pools, half, qki, qt_b, kt_b,
                                    p_b, h, grp=g,
                                )
                                qki += 1
                        g = next_grp()
                        n = 5
                        while n > 0 and jobs:
                            take = min(n, KT - jobs[0].k)
                            jobs[0].step(nc, tc, pools, aps, take, grp=g)
                            if jobs[0].k > KT:
                                jobs.pop(0)
                            n -= take
                    g = next_grp()
                    for j in jobs:
                        j.step(nc, tc, pools, aps, KT, grp=g)
                prev = (p_b, v_b)
            g = next_grp()
            for qc in (2, 3):
                PVChunk(HPC - 1, *prev, qc).step(nc, tc, pools, aps, KT, grp=g)

    nc.compile()
    return nc


def shard_inputs(Q, K, V):
    """Full [B,H,S,D] f32 -> per-core input maps (layout + dtype choices)."""
    Qh = np.asarray(Q, dtype=np.float32).reshape(B * H, S, D)
    Kh = np.asarray(K, dtype=np.float32).reshape(B * H, S, D)
    Vh = np.asarray(V, dtype=np.float32).reshape(B * H, S, D)

    in_maps = []
    for c in range(NCORES):
        sl = slice(c * HPC, (c + 1) * HPC)
        qt = np.empty((HPC, P, S), dtype=ml_dtypes.bfloat16)
        kt = np.empty((HPC, P, S), dtype=ml_dtypes.bfloat16)
        qt[:, :D, :] = Qh[sl].transpose(0, 2, 1).astype(ml_dtypes.bfloat16)
        kt[:, :D, :] = Kh[sl].transpose(0, 2, 1).astype(ml_dtypes.bfloat16)
        qt[:, D:, :] = qt[:, :D, :]  # duplicate for row-group 64-127
        kt[:, D:, :] = kt[:, :D, :]
        vp = np.ones((HPC, S, D + 1), dtype=np.float32)
        vp[:, :, :D] = Vh[sl]
        # [h, (kt p), d] -> [h, p, kt, d']
        vp = (
            vp.reshape(HPC, KT, P, D + 1)
            .transpose(0, 2, 1, 3)
            .astype(ml_dtypes.bfloat16)
        )
        in_maps.append({"qt": np.ascontiguousarray(qt),
                        "kt": np.ascontiguousarray(kt),
                        "vp": np.ascontiguousarray(vp)})
    return in_maps


_NC_CACHE = None


def unshard_outputs(res):
    out = np.empty((B * H, S, D), dtype=np.float32)
    for c in range(NCORES):
        o = res.results[c]["ot"]  # [HPC, D+1, S] unnormalized, transposed
        out[c * HPC : (c + 1) * HPC] = (
            o[:, :D, :] / o[:, D : D + 1, :]
        ).transpose(0, 2, 1)
    return out.reshape(B, H, S, D)


def kernel(Q, K, V):
    global _NC_CACHE
    if _NC_CACHE is None:
        _NC_CACHE = build_nc()
    nc = _NC_CACHE
    in_maps = shard_inputs(Q, K, V)
    res = run_bass_kernel_spmd(nc, in_maps, core_ids=list(range(NCORES)))
    return unshard_outputs(res)


if __name__ == "__main__":
    nc = build_nc()
    print("compiled OK")
